# revision 32
# baseline (speedup 1.0000x reference)
"""Tensor-parallel causal attention kernel for 8 trn2 NeuronCores.

Problem: B=2, S=2048, H=2048, 16 heads, head_dim=128 fp32.
  qkv = hidden @ w_qkv.T ; causal attention ; out = attn @ w_o.T

Sharding (hardcoded): core c in 0..7 handles batch b=c//4 and heads
hs = [4*(c%4) .. 4*(c%4)+3].  Each core computes a partial o_proj
output (contraction over its 512 hidden dims); the host sums the 4
partials per batch and transposes.  No device collectives.

Design (fp16 everywhere; fp8 ruled out: DoubleRow LDW serialization
erases the rate win at 512-wide outputs, and e4m3 quantization of
q/k/v measures ~1-3e-2 worst-case output error vs the 2e-2 gate):
  * All matmul operands fp16 (same 1 cycle/row PE rate as bf16 with
    ~16x less rounding noise); est in bf16; psum f32.
  * Causal diagonal blocks narrowed: ST/exp/mask/PV only touch the
    valid q-column range (saves ~20%% of attention work).
  * Softmax denominators WITHOUT per-block rowsum matmuls: colsum
    (bf16) accumulates est blocks on DVE/GpSimd, then ONE all-ones
    [128x128] matmul per unit both partition-reduces colsum and
    broadcasts the denominator to 128 rows; reciprocal in one
    approximate custom-DVE op (~51 ULP), normalize on DVE into fp16
    attnT.  (The old per-block ones-matmul rowsum burned ~10%% of the
    tensor engine and its open skip-group accumulation added ~96ns to
    every narrow diagonal matmul.)
  * Engine balance in the attention phase (ACT owns all exps at
    (N+352)/1.2 ns each): colsum for non-diagonal blocks on GpSimd
    (otherwise idle), diagonal colsum on DVE (avoids serializing
    behind its own affine_select in the GpSimd queue), o_proj
    psum->sbuf stages split 3:1 DVE:ACT (GpSimd cannot read PSUM).
  * Emission is software-pipelined around the in-order engine queues:
    score matmuls run 3 blocks ahead of their exp; normalization
    chains and o_proj groups go into deferred queues drained between
    attention blocks so their tensor ops fill pipeline gaps.
  * Attention units are generators pumped between phase-1 projection
    groups, so attention's exp/mask latency hides under phase 1's
    saturated tensor stream.
  * Lead-in: x is chunk-major in dram (contiguous 16KB/partition DMA
    runs), chunk 0 is processed in two 256-token halves so the first
    accumulation group waits for ~1MB of DMA instead of 4MB, and the
    initial loads fan out across the Sync/GpSimd/ACT DMA queues
    (each DMA_DIRECT2D dispatch costs ~600-850ns of queue time).
  * PSUM banks: p1 2 + score/rep 4 + attn-accum 2 = 8 in phase 1;
    score/rep 4 + attn-accum 2 + o_proj 2 = 8 after.
  * SBUF tile pools padded so matmul operands stay 256B-aligned.
  * fp16 output partials (half the store DMA); host sums in f32.

Device layout (host-pretiled, partition-major):
  xt  [128,4,16,512] f16 : xt[p,xc,ko,s'] = hidden[b, xc*512+s', ko*128+p]
  wq  [128,16, 512] f16 : wq[p,ko,o]  = w_qkv[q_rows[o],  ko*128+p]
  wk, wv same as wq (k_rows / v_rows)
  wo  [128, 4,2048] f16 : wo[p,kb,o]  = w_o[o, cols[kb*128+p]]
  outt[128,16,2048] f16 : outt[p,ot,s] = outT_partial[ot*128+p, s]

Measurement notes: the device thermal-throttles under sustained load
(cold 2.4GHz -> warm 2.0GHz -> hot 2.0GHz with K=4/8 oscillation), so
A/B timings are only comparable back-to-back at similar temperature.

Toolchain quirks worked around here (walrus 1-sync-wait slots):
  - chunked tail drain monkeypatch; NoOp splitting of multi-waits
  - GPSIMD cannot access PSUM (BIR verifier rejects)
  - lower_extended_insts() for custom-DVE ISA instruction bytes
"""
import numpy as np

import concourse.bass as bass
import concourse.mybir as mybir
import concourse.tile as tile
from concourse.bass_utils import run_bass_kernel_spmd
from concourse.vector_clock import ScopedClock, VectorClock

P = 128
S = 2048
H = 2048
NH_LOCAL = 4          # heads per core
KO = H // P           # 16 contraction chunks for the projections
SQ = 512              # q chunk width
NQC = S // SQ         # 4 q chunks
NKB = S // P          # 16 key blocks
F32 = mybir.dt.float32
F32R = mybir.dt.float32r
F16 = mybir.dt.float16
BF = mybir.dt.bfloat16
AF = mybir.ActivationFunctionType
SCALE = 1.0 / float(np.sqrt(128.0))

XCH = 512             # x chunk width in phase 1
NXCH = S // XCH       # 4 chunks


def _drain_and_barrier_chunked(self, tick_clock, wait_clock, _MAX=1):
    """Split the kernel-tail drain's waits: walrus allows only one sync
    wait per instruction in this toolchain."""
    g = tick_clock.global_clock
    n = len(g)
    vals = [g[i] for i in range(n)]
    nz = [i for i, v in enumerate(vals) if v > 0]
    chunks = [nz[i:i + _MAX] for i in range(0, len(nz), _MAX)] or [[]]
    for chunk in chunks:
        vec = [vals[i] if i in chunk else 0 for i in range(n)]
        d = self.nc.sync.drain()
        wait_clock.add_sem_waits(d.ins, ScopedClock({None: VectorClock(vec)}))
    self.nc.all_engine_barrier()
    assert self.sems is not None
    popped = self.nc._tile_sem_poison_stack.pop()
    assert popped is self._sem_poison
    self.nc.clear_and_free_semaphores(list(self.sems.allocated().values()))
    self.nc.all_engine_barrier()


tile.TileContext._drain_and_barrier = _drain_and_barrier_chunked


def _split_multi_waits(nc):
    """walrus allows ONE sync wait per instruction: hoist extra waits onto
    same-engine NoOps inserted directly before the offending instruction
    (identical semantics — the engine queue blocks on each in turn)."""
    ctr = 0
    for f in nc.m.functions:
        for blk in f.blocks:
            new = []
            changed = False
            for inst in blk.instructions:
                si = inst.sync_info
                waits = list(si.on_wait) if si and si.on_wait else []
                if len(waits) > 1:
                    changed = True
                    for w in waits[:-1]:
                        ctr += 1
                        nop = mybir.InstNoOp(name=f"I-wsplit-{ctr}",
                                             engine=inst.engine,
                                             ins=[], outs=[])
                        nop.sync_info = mybir.SyncInfo(on_wait=[w],
                                                       on_update=[])
                        new.append(nop)
                    ups = list(si.on_update) if si.on_update else []
                    inst.sync_info = mybir.SyncInfo(on_wait=[waits[-1]],
                                                   on_update=ups)
                new.append(inst)
            if changed:
                blk.instructions = new
    return ctr


def build():
    nc = bass.Bass()
    # x chunk-major: [p, xc, ko, s'] so each chunk's DMA is one contiguous
    # 16KB-per-partition run (big DMA packets, fast spool-up)
    xt = nc.dram_tensor("xt", [P, NXCH, KO, XCH], F16, kind="ExternalInput")
    # duplicate of the first 256 tokens, ko-contiguous per partition: the
    # lead-in-critical DMA lands in 2KB runs instead of 512B
    xh0 = nc.dram_tensor("xh0", [P, KO, XCH // 2], F16, kind="ExternalInput")
    wq = nc.dram_tensor("wq", [P, KO, NH_LOCAL * P], F16, kind="ExternalInput")
    wk = nc.dram_tensor("wk", [P, KO, NH_LOCAL * P], F16, kind="ExternalInput")
    wv = nc.dram_tensor("wv", [P, KO, NH_LOCAL * P], F16, kind="ExternalInput")
    wo = nc.dram_tensor("wo", [P, NH_LOCAL, S], F16, kind="ExternalInput")
    outt = nc.dram_tensor("outt", [P, KO, S], F16, kind="ExternalOutput")

    with tile.TileContext(nc) as tc:
        from contextlib import ExitStack
        with ExitStack() as ctx:
            const = ctx.enter_context(tc.tile_pool(name="const", bufs=1))

            # ---- constants -------------------------------------------------
            # all-ones [128,128] f32r stationary: one matmul per unit both
            # partition-reduces colsum AND broadcasts the denominator to all
            # 128 rows (replaces per-block rowsum matmuls + rank-1 rep).
            # pads: keep ones128_r AND all downstream pools 256B-aligned
            # (matmul operands at 64B-misaligned SBUF addresses stream ~35%
            # slower: 454ns vs 379ns per 512-wide fp16 matmul, measured)
            pad_head = const.tile([P, 32], F32)  # noqa: F841 (128B/part)
            ones_f = const.tile([P, P], F32)
            nc.vector.memset(ones_f[:], 1.0)
            ones128_b = const.tile([P, P], BF)
            nc.scalar.copy(ones128_b[:], ones_f[:])
            pad_tail = const.tile([P, 112], F32)  # noqa: F841 (448B/part)
            obs_dve = const.tile([1, 1], F32)
            nc.vector.memset(obs_dve[:], 0.0)
            obs_act = const.tile([1, 1], F32)
            nc.vector.memset(obs_act[:], 0.0)

            def _one(ap):
                return ap[tuple(slice(0, 1) for _ in ap.shape)]

            def dve_war_touch(ap):
                nc.vector.tensor_copy(_one(ap[:]), obs_dve[:])

            # ---- residents (per 512-token chunk so attention reads never
            # false-serialize against later-chunk projection writes) -------
            qkv_pool = ctx.enter_context(tc.tile_pool(name="qkvp", bufs=1))
            # Q,K per chunk: [d_in, o_tile(0-3 Q heads, 4-7 K heads), 512]
            qk_c = [qkv_pool.tile([P, 2 * NH_LOCAL, XCH], F16,
                                  name=f"qkc{i}") for i in range(NXCH)]
            # V per chunk: [s_in, s_tile(4), d_local(512)]
            v_c = [qkv_pool.tile([P, 4, NH_LOCAL * P], F16,
                                 name=f"vc{i}") for i in range(NXCH)]
            # attn per q chunk: [d_local, head, 512] fp16
            attn_c = [qkv_pool.tile([P, NH_LOCAL, SQ], F16,
                                    name=f"attnc{i}") for i in range(NQC)]

            # ---- attention pools (coexist with phase 1: 2 p1 + 4 st +
            # 2 at = 8 psum banks; p3 opens after phase 1) ------------------
            p2sb = ctx.enter_context(tc.tile_pool(name="p2sb", bufs=4))
            p2est = ctx.enter_context(tc.tile_pool(name="p2est", bufs=12))
            p2cs = ctx.enter_context(tc.tile_pool(name="p2cs", bufs=3))
            p2st = ctx.enter_context(
                tc.tile_pool(name="p2st", bufs=4, space="PSUM"))
            p2at = ctx.enter_context(
                tc.tile_pool(name="p2at", bufs=2, space="PSUM"))
            p3ps = None    # assigned after phase 1 (bank budget)
            p3sb = None

            pending_norm = []
            pending_oproj = []
            slot = [0]
            in_p1 = [True]

            def drain_slot():
                slot[0] += 1
                while pending_norm:
                    pending_norm.pop(0)()     # norms gate psum reuse: ASAP
                if in_p1[0]:
                    return
                if pending_oproj and (len(pending_oproj) > 16
                                      or slot[0] % 2 == 0):
                    pending_oproj.pop(0)()

            def make_norm(at_ps, colsum, h, qc):
                def norm():
                    rep_ps = p2st.tile([P, SQ], F32, tag="stps",
                                       name="rep_ps")
                    nc.tensor.matmul(rep_ps[:], ones128_b[:], colsum[:],
                                     start=True, stop=True)
                    rep_sb = p2sb.tile([P, SQ], F32, tag="repsb")
                    nc.vector.reciprocal_approx_fast(rep_sb[:], rep_ps[:])
                    nc.vector.tensor_mul(attn_c[qc][:, h, :],
                                         at_ps[:], rep_sb[:])
                return norm

            def make_oproj(sc, ot):
                # two half-groups drained on consecutive slots: 432ns
                # tensor bursts interleave with attention blocks more
                # smoothly than a monolithic 864ns group
                state = {}

                def half_a():
                    ps = p3ps.tile([P, SQ], F32, tag="p3ps")
                    state["ps"] = ps
                    for kb in (0, 1):
                        nc.tensor.matmul(
                            ps[:], wo_r[:, kb, ot * P:(ot + 1) * P],
                            attn_c[sc][:, kb, :],
                            start=(kb == 0), stop=False)

                def half_b():
                    ps = state["ps"]
                    for kb in (2, 3):
                        nc.tensor.matmul(
                            ps[:], wo_r[:, kb, ot * P:(ot + 1) * P],
                            attn_c[sc][:, kb, :],
                            start=False, stop=(kb == 3))
                    # psum->sbuf f16 stage copies: GpSimd can't read PSUM,
                    # so split 3:1 DVE:ACT (ACT is the busier engine here —
                    # it owns all the exps)
                    stage = p3sb.tile([P, SQ], F16, tag="p3stage")
                    if ot % 4 == 0:
                        nc.scalar.copy(_one(stage[:]), obs_act[:])
                        nc.scalar.copy(stage[:], ps[:])
                    else:
                        dve_war_touch(stage)
                        nc.vector.tensor_copy(stage[:], ps[:])
                    nc.sync.dma_start(
                        outt.ap()[:, ot, sc * SQ:(sc + 1) * SQ],
                        stage[:])
                return half_a, half_b

            def att_unit(h, qc):
                """Generator: one causal-attention unit, yielding after
                each key-block so it can be pumped between phase-1
                projection groups (whose matmuls hide the exp/mask
                latency)."""
                nkb = 4 * (qc + 1)
                qs = qc * SQ
                at_ps = p2at.tile([P, SQ], F32, tag="atps")
                colsum = p2cs.tile([P, SQ], BF, tag="colsum")
                st_tiles = {}

                def off_of(kb):
                    return max(0, kb * P - qs)

                def emit_st(kb):
                    st_ps = p2st.tile([P, SQ], F32, tag="stps")
                    off = off_of(kb)
                    nc.tensor.matmul(
                        st_ps[:, off:SQ],
                        qk_c[kb // 4][:, NH_LOCAL + h,
                                      (kb % 4) * P:(kb % 4 + 1) * P],
                        qk_c[qc][:, h, off:SQ],
                        start=True, stop=True)
                    st_tiles[kb] = st_ps

                emit_st(0)
                emit_st(1)
                if nkb > 2:
                    emit_st(2)
                for kb in range(nkb):
                    drain_slot()
                    if kb + 3 < nkb:
                        emit_st(kb + 3)
                    st_ps = st_tiles.pop(kb)
                    off = off_of(kb)
                    est = p2est.tile([P, SQ], BF, tag="est")
                    nc.scalar.activation(est[:, off:SQ], st_ps[:, off:SQ],
                                         AF.Exp, scale=SCALE)
                    if kb * P + P - 1 > qs:  # crosses the causal diagonal
                        nc.gpsimd.affine_select(
                            est[:, off:SQ], est[:, off:SQ], [[1, SQ - off]],
                            mybir.AluOpType.is_ge, 0.0,
                            base=qs + off - kb * P,
                            channel_multiplier=-1)
                    # colsum accumulation: non-diagonal blocks go to GpSimd
                    # (mostly idle); diagonal blocks stay on DVE so they
                    # don't serialize behind their own affine_select in the
                    # GpSimd queue
                    diag = kb * P + P - 1 > qs
                    if kb == 0:
                        eng = nc.gpsimd if not diag else nc.vector
                        eng.tensor_copy(colsum[:], est[:])
                    elif diag:
                        nc.vector.tensor_add(colsum[:, off:SQ],
                                             colsum[:, off:SQ],
                                             est[:, off:SQ])
                    else:
                        nc.gpsimd.tensor_add(colsum[:, off:SQ],
                                             colsum[:, off:SQ],
                                             est[:, off:SQ])
                    nc.tensor.matmul(
                        at_ps[:, off:SQ],
                        v_c[kb // 4][:, kb % 4, h * P:(h + 1) * P],
                        est[:, off:SQ],
                        start=(kb == 0), stop=(kb == nkb - 1))
                    yield
                pending_norm.append(make_norm(at_ps, colsum, h, qc))

            gens = [(h, qc, att_unit(h, qc))
                    for qc in range(NQC) for h in range(NH_LOCAL)]
            gen_idx = [0]

            def pump(max_qc):
                """Advance the attention emission by one key-block."""
                while gen_idx[0] < len(gens):
                    h, qc, g = gens[gen_idx[0]]
                    if qc >= max_qc:
                        return False
                    try:
                        next(g)
                        return True
                    except StopIteration:
                        if h == NH_LOCAL - 1:
                            for ot in range(KO):
                                pending_oproj.extend(make_oproj(qc, ot))
                        gen_idx[0] += 1
                return False

            # ================= phase 1: QKV projection =====================
            # fp16 matmuls, one pass over x in 512-token chunks.  After
            # each projection group, attention units whose inputs are
            # ready are pumped in (their exp/mask latency hides under the
            # next group's matmuls).  Chunk 0 is processed in two
            # 256-token halves so the first accumulation group only waits
            # for ~1MB of DMA, not 4MB.
            # w free layout: [0:512]=Q, [512:1024]=K, [1024:1536]=V
            p3w = ctx.enter_context(tc.tile_pool(name="p3w", bufs=1))
            wo_r = p3w.tile([P, NH_LOCAL, S], F16)
            with tc.tile_pool(name="p1w", bufs=1) as p1w, \
                 tc.tile_pool(name="p1x", bufs=2) as p1x, \
                 tc.tile_pool(name="p1ps", bufs=2, space="PSUM") as p1ps:

                w_r = p1w.tile([P, KO, 3 * NH_LOCAL * P], F16, tag="wr")
                x_tiles = []
                x_r0 = p1x.tile([P, KO, XCH], F16, tag="xr", name="xr0")
                # per-ko staging so the first accumulation group can
                # start as soon as ko-chunk 0 has landed (first half of
                # chunk 0 only; second half follows as one DMA)
                # initial loads fan out across engine queues (each
                # DMA_DIRECT2D dispatch costs ~600-850ns of queue time) and
                # only the lead-in-critical tiles go now: the first-half x
                # (contiguous xh0 copy) + wq.  wk/wv/x-2nd-half/wo are
                # dispatched behind the first projection groups below so
                # they don't steal DMA bandwidth from the critical path.
                HX = XCH // 2
                for kq in range(8):
                    ks = slice(2 * kq, 2 * (kq + 1))
                    nc.sync.dma_start(x_r0[:, ks, 0:HX], xh0.ap()[:, ks])
                    nc.gpsimd.dma_start(w_r[:, ks, 0:4 * P], wq.ap()[:, ks])
                nc.scalar.dma_start(w_r[:, :, 4 * P:8 * P], wk.ap())
                nc.scalar.dma_start(w_r[:, :, 8 * P:12 * P], wv.ap())
                nc.scalar.dma_start(x_r0[:, :, HX:XCH],
                                    xt.ap()[:, 0, :, HX:XCH])
                nc.gpsimd.dma_start(wo_r[:], wo.ap())
                x_tiles.append(x_r0)

                def qk_group(xc, x_r, ot, lo, hi):
                    ps = p1ps.tile([P, XCH], F32, tag="p1", name="ps")
                    for k in range(KO):
                        nc.tensor.matmul(
                            ps[:, 0:hi - lo], w_r[:, k, ot * P:(ot + 1) * P],
                            x_r[:, k, lo:hi], start=(k == 0),
                            stop=(k == KO - 1))
                    if ot % 2 == 0:
                        nc.scalar.copy(qk_c[xc][:, ot, lo:hi],
                                       ps[:, 0:hi - lo])
                    else:
                        nc.vector.tensor_copy(qk_c[xc][:, ot, lo:hi],
                                              ps[:, 0:hi - lo])
                    pump(xc)
                    pump(xc)

                def v_group(xc, x_r, st):
                    # V: out [s_tile(128), d(512)] — copies on ACT
                    ps = p1ps.tile([P, NH_LOCAL * P], F32, tag="p1",
                                   name="ps")
                    for k in range(KO):
                        nc.tensor.matmul(
                            ps[:], x_r[:, k, st * P:(st + 1) * P],
                            w_r[:, k, 2 * NH_LOCAL * P:3 * NH_LOCAL * P],
                            start=(k == 0), stop=(k == KO - 1))
                    nc.scalar.copy(v_c[xc][:, st, :], ps[:])
                    pump(xc)
                    pump(xc)

                for xc in range(NXCH):
                    if xc > 0:
                        x_r = p1x.tile([P, KO, XCH], F16, tag="xr")
                        nc.sync.dma_start(x_r[:], xt.ap()[:, xc])
                    else:
                        x_r = x_tiles[0]

                    if xc == 0:
                        for half in range(2):
                            lo, hi = half * HX, (half + 1) * HX
                            for ot in range(2 * NH_LOCAL):
                                qk_group(xc, x_r, ot, lo, hi)
                            for st in (2 * half, 2 * half + 1):
                                v_group(xc, x_r, st)
                    else:
                        for ot in range(2 * NH_LOCAL):
                            qk_group(xc, x_r, ot, 0, XCH)
                        for st in range(XCH // P):
                            v_group(xc, x_r, st)

            # ============ phase 2+3: remaining attention + o_proj ==========
            in_p1[0] = False
            p3ps = ctx.enter_context(
                tc.tile_pool(name="p3ps", bufs=2, space="PSUM"))
            p3sb = ctx.enter_context(tc.tile_pool(name="p3sb", bufs=4))

            while pump(NQC):
                pass
            while pending_norm:
                pending_norm.pop(0)()
            while pending_oproj:
                pending_oproj.pop(0)()

    from concourse.library_overlay import lower_extended_insts
    lower_extended_insts(nc)   # populate .instr bytes for custom ISA ops
    _split_multi_waits(nc)
    return nc


_NC_CACHE = None


def _get_nc():
    global _NC_CACHE
    if _NC_CACHE is None:
        _NC_CACHE = build()
    return _NC_CACHE


def _prep_inputs(hidden_states, w_qkv, w_o):
    """Host-side shard + pre-tile + fp16-cast for the 8 cores."""
    F16_NP = np.float16
    hidden_states = np.asarray(hidden_states, dtype=np.float32)
    w_qkv = np.asarray(w_qkv, dtype=np.float32)
    w_o = np.asarray(w_o, dtype=np.float32)
    B = hidden_states.shape[0]

    in_maps = []
    xt_by_b = {}
    xh0_by_b = {}
    for b in range(B):
        # xt[p, xc, ko, s'] = hidden[b, xc*512+s', ko*128+p]
        xt_by_b[b] = np.ascontiguousarray(
            hidden_states[b].T.reshape(KO, P, NXCH, XCH)
            .transpose(1, 2, 0, 3)
        ).astype(F16_NP)
        # first 256 tokens again, ko-contiguous (lead-in DMA)
        xh0_by_b[b] = np.ascontiguousarray(
            xt_by_b[b][:, 0, :, 0:XCH // 2])
    for c in range(8):
        b = c // 4
        hs = [4 * (c % 4) + j for j in range(NH_LOCAL)]
        q_rows = np.concatenate([np.arange(h * P, (h + 1) * P) for h in hs])
        k_rows = q_rows + H
        v_rows = q_rows + 2 * H

        def wtile(rows):
            # [p, ko, o] = w_qkv[rows[o], ko*128+p]
            w = w_qkv[rows, :]                      # [512, 2048]
            return np.ascontiguousarray(
                w.T.reshape(KO, P, len(rows)).transpose(1, 0, 2)
            ).astype(F16_NP)

        # wo[p, kb, o] = w_o[o, cols[kb*128+p]]
        wo_c = np.ascontiguousarray(
            w_o[:, q_rows].T.reshape(NH_LOCAL, P, S).transpose(1, 0, 2)
        ).astype(F16_NP)
        in_maps.append({
            "xt": xt_by_b[b],
            "xh0": xh0_by_b[b],
            "wq": wtile(q_rows),
            "wk": wtile(k_rows),
            "wv": wtile(v_rows),
            "wo": wo_c,
        })
    return in_maps


def run(hidden_states, w_qkv, w_o, trace=False, trace_cores=None):
    in_maps = _prep_inputs(hidden_states, w_qkv, w_o)
    nc = _get_nc()
    kwargs = {}
    if trace:
        kwargs["trace_cores"] = (trace_cores if trace_cores is not None
                                 else list(range(8)))
    res = run_bass_kernel_spmd(nc, in_maps, core_ids=list(range(8)),
                               trace=trace, **kwargs)
    B, S_, H_ = np.asarray(hidden_states).shape
    out = np.zeros((B, S_, H_), dtype=np.float32)
    for c in range(8):
        b = c // 4
        outt = res.results[c]["outt"]               # [128, 16, 2048] fp16
        outT = outt.astype(np.float32).transpose(1, 0, 2).reshape(H_, S_)
        out[b] += outT.T
    return out, res


def kernel(hidden_states, w_qkv, w_o):
    out, _ = run(hidden_states, w_qkv, w_o, trace=False)
    return out



# revision 34
# speedup vs baseline: 1.1068x; 1.1068x over previous
"""Tensor-parallel causal attention kernel for 8 trn2 NeuronCores.

Problem: B=2, S=2048, H=2048, 16 heads, head_dim=128 fp32.
  qkv = hidden @ w_qkv.T ; causal attention ; out = attn @ w_o.T

Sharding (hardcoded): core c in 0..7 handles batch b=c//4 and heads
hs = [4*(c%4) .. 4*(c%4)+3].  Each core computes a partial o_proj
output (contraction over its 512 hidden dims); the host sums the 4
partials per batch and transposes.  No device collectives.

Design (fp16 everywhere; fp8 ruled out: DoubleRow LDW serialization
erases the rate win at 512-wide outputs, and e4m3 quantization of
q/k/v measures ~1-3e-2 worst-case output error vs the 2e-2 gate):
  * All matmul operands fp16 (same 1 cycle/row PE rate as bf16 with
    ~16x less rounding noise); est in bf16; psum f32.
  * Causal diagonal blocks narrowed: ST/exp/mask/PV only touch the
    valid q-column range (saves ~20%% of attention work).
  * Softmax denominators WITHOUT per-block rowsum matmuls: colsum
    (bf16) accumulates est blocks on DVE/GpSimd, then ONE all-ones
    [128x128] matmul per unit both partition-reduces colsum and
    broadcasts the denominator to 128 rows; reciprocal in one
    approximate custom-DVE op (~51 ULP), normalize on DVE into fp16
    attnT.  (The old per-block ones-matmul rowsum burned ~10%% of the
    tensor engine and its open skip-group accumulation added ~96ns to
    every narrow diagonal matmul.)
  * Engine balance in the attention phase (ACT owns all exps at
    (N+352)/1.2 ns each): colsum for non-diagonal blocks on GpSimd
    (otherwise idle), diagonal colsum on DVE (avoids serializing
    behind its own affine_select in the GpSimd queue), o_proj
    psum->sbuf stages split 3:1 DVE:ACT (GpSimd cannot read PSUM).
  * Emission is software-pipelined around the in-order engine queues:
    score matmuls run 3 blocks ahead of their exp; normalization
    chains and o_proj groups go into deferred queues drained between
    attention blocks so their tensor ops fill pipeline gaps.
  * Attention units are generators pumped between phase-1 projection
    groups, so attention's exp/mask latency hides under phase 1's
    saturated tensor stream.
  * Lead-in: x is chunk-major in dram (contiguous 16KB/partition DMA
    runs), chunk 0 is processed in two 256-token halves so the first
    accumulation group waits for ~1MB of DMA instead of 4MB, and the
    initial loads fan out across the Sync/GpSimd/ACT DMA queues
    (each DMA_DIRECT2D dispatch costs ~600-850ns of queue time).
  * PSUM banks: p1 2 + score/rep 4 + attn-accum 2 = 8 in phase 1;
    score/rep 4 + attn-accum 2 + o_proj 2 = 8 after.
  * SBUF tile pools padded so matmul operands stay 256B-aligned.
  * fp16 output partials (half the store DMA); host sums in f32.

Device layout (host-pretiled, partition-major):
  xt  [128,4,16,512] f16 : xt[p,xc,ko,s'] = hidden[b, xc*512+s', ko*128+p]
  wq  [128,16, 512] f16 : wq[p,ko,o]  = w_qkv[q_rows[o],  ko*128+p]
  wk, wv same as wq (k_rows / v_rows)
  wo  [128, 4,2048] f16 : wo[p,kb,o]  = w_o[o, cols[kb*128+p]]
  outt[128,16,2048] f16 : outt[p,ot,s] = outT_partial[ot*128+p, s]

Measurement notes: the device thermal-throttles under sustained load
(cold 2.4GHz -> warm 2.0GHz -> hot 2.0GHz with K=4/8 oscillation), so
A/B timings are only comparable back-to-back at similar temperature.

Toolchain quirks worked around here (walrus 1-sync-wait slots):
  - chunked tail drain monkeypatch; NoOp splitting of multi-waits
  - GPSIMD cannot access PSUM (BIR verifier rejects)
  - lower_extended_insts() for custom-DVE ISA instruction bytes
"""
import numpy as np

import concourse.bass as bass
import concourse.mybir as mybir
import concourse.tile as tile
from concourse.bass_utils import run_bass_kernel_spmd
from concourse.vector_clock import ScopedClock, VectorClock

P = 128
S = 2048
H = 2048
NH_LOCAL = 4          # heads per core
KO = H // P           # 16 contraction chunks for the projections
SQ = 512              # q chunk width
NQC = S // SQ         # 4 q chunks
NKB = S // P          # 16 key blocks
F32 = mybir.dt.float32
F32R = mybir.dt.float32r
F16 = mybir.dt.float16
BF = mybir.dt.bfloat16
AF = mybir.ActivationFunctionType
SCALE = 1.0 / float(np.sqrt(128.0))

XCH = 512             # x chunk width in phase 1
NXCH = S // XCH       # 4 chunks


def _drain_and_barrier_chunked(self, tick_clock, wait_clock, _MAX=1):
    """Split the kernel-tail drain's waits: walrus allows only one sync
    wait per instruction in this toolchain."""
    g = tick_clock.global_clock
    n = len(g)
    vals = [g[i] for i in range(n)]
    nz = [i for i, v in enumerate(vals) if v > 0]
    chunks = [nz[i:i + _MAX] for i in range(0, len(nz), _MAX)] or [[]]
    for chunk in chunks:
        vec = [vals[i] if i in chunk else 0 for i in range(n)]
        d = self.nc.sync.drain()
        wait_clock.add_sem_waits(d.ins, ScopedClock({None: VectorClock(vec)}))
    self.nc.all_engine_barrier()
    assert self.sems is not None
    popped = self.nc._tile_sem_poison_stack.pop()
    assert popped is self._sem_poison
    self.nc.clear_and_free_semaphores(list(self.sems.allocated().values()))
    self.nc.all_engine_barrier()


tile.TileContext._drain_and_barrier = _drain_and_barrier_chunked


def _split_multi_waits(nc):
    """walrus allows ONE sync wait per instruction: hoist extra waits onto
    same-engine NoOps inserted directly before the offending instruction
    (identical semantics — the engine queue blocks on each in turn)."""
    ctr = 0
    for f in nc.m.functions:
        for blk in f.blocks:
            new = []
            changed = False
            for inst in blk.instructions:
                si = inst.sync_info
                waits = list(si.on_wait) if si and si.on_wait else []
                if len(waits) > 1:
                    changed = True
                    for w in waits[:-1]:
                        ctr += 1
                        nop = mybir.InstNoOp(name=f"I-wsplit-{ctr}",
                                             engine=inst.engine,
                                             ins=[], outs=[])
                        nop.sync_info = mybir.SyncInfo(on_wait=[w],
                                                       on_update=[])
                        new.append(nop)
                    ups = list(si.on_update) if si.on_update else []
                    inst.sync_info = mybir.SyncInfo(on_wait=[waits[-1]],
                                                   on_update=ups)
                new.append(inst)
            if changed:
                blk.instructions = new
    return ctr


def build():
    nc = bass.Bass()
    # x chunk-major: [p, xc, ko, s'] so each chunk's DMA is one contiguous
    # 16KB-per-partition run (big DMA packets, fast spool-up)
    xt = nc.dram_tensor("xt", [P, NXCH, KO, XCH], F16, kind="ExternalInput")
    # duplicate of the first 256 tokens, ko-contiguous per partition: the
    # lead-in-critical DMA lands in 2KB runs instead of 512B
    xh0 = nc.dram_tensor("xh0", [P, KO, XCH // 2], F16, kind="ExternalInput")
    wq = nc.dram_tensor("wq", [P, KO, NH_LOCAL * P], F16, kind="ExternalInput")
    wk = nc.dram_tensor("wk", [P, KO, NH_LOCAL * P], F16, kind="ExternalInput")
    wv = nc.dram_tensor("wv", [P, KO, NH_LOCAL * P], F16, kind="ExternalInput")
    wo = nc.dram_tensor("wo", [P, NH_LOCAL, S], F16, kind="ExternalInput")
    outt = nc.dram_tensor("outt", [P, KO, S], F16, kind="ExternalOutput")

    with tile.TileContext(nc) as tc:
        from contextlib import ExitStack
        with ExitStack() as ctx:
            const = ctx.enter_context(tc.tile_pool(name="const", bufs=1))

            # ---- constants -------------------------------------------------
            # all-ones [128,128] f32r stationary: one matmul per unit both
            # partition-reduces colsum AND broadcasts the denominator to all
            # 128 rows (replaces per-block rowsum matmuls + rank-1 rep).
            # pads: keep ones128_r AND all downstream pools 256B-aligned
            # (matmul operands at 64B-misaligned SBUF addresses stream ~35%
            # slower: 454ns vs 379ns per 512-wide fp16 matmul, measured)
            pad_head = const.tile([P, 32], F32)  # noqa: F841 (128B/part)
            ones_f = const.tile([P, P], F32)
            nc.vector.memset(ones_f[:], 1.0)
            ones128_b = const.tile([P, P], BF)
            nc.scalar.copy(ones128_b[:], ones_f[:])
            # causal mask: every diagonal block's affine_select has base=0
            # (off = (kb-4qc)*128 exactly cancels qs - kb*128), so ONE
            # triangular tile M[p,c] = (c >= p) serves all of them as a DVE
            # multiply -- keeps the tensor-critical est path off the slow,
            # congested GpSimd queue (1150ns/colsum-add there)
            ones512_b = const.tile([P, SQ], BF)
            nc.vector.memset(ones512_b[:], 1.0)
            mask_t = const.tile([P, SQ], BF)
            nc.gpsimd.affine_select(
                mask_t[:], ones512_b[:], [[1, SQ]],
                mybir.AluOpType.is_ge, 0.0, base=0, channel_multiplier=-1)
            pad_tail = const.tile([P, 112], F32)  # noqa: F841 (448B/part)
            obs_dve = const.tile([1, 1], F32)
            nc.vector.memset(obs_dve[:], 0.0)
            obs_act = const.tile([1, 1], F32)
            nc.vector.memset(obs_act[:], 0.0)

            def _one(ap):
                return ap[tuple(slice(0, 1) for _ in ap.shape)]

            def dve_war_touch(ap):
                nc.vector.tensor_copy(_one(ap[:]), obs_dve[:])

            # ---- residents (per 512-token chunk so attention reads never
            # false-serialize against later-chunk projection writes) -------
            qkv_pool = ctx.enter_context(tc.tile_pool(name="qkvp", bufs=1))
            # Q,K per chunk: [d_in, o_tile(0-3 Q heads, 4-7 K heads), 512]
            qk_c = [qkv_pool.tile([P, 2 * NH_LOCAL, XCH], F16,
                                  name=f"qkc{i}") for i in range(NXCH)]
            # V per chunk: [s_in, s_tile(4), d_local(512)]
            v_c = [qkv_pool.tile([P, 4, NH_LOCAL * P], F16,
                                 name=f"vc{i}") for i in range(NXCH)]
            # attn per q chunk: [d_local, head, 512] fp16
            attn_c = [qkv_pool.tile([P, NH_LOCAL, SQ], F16,
                                    name=f"attnc{i}") for i in range(NQC)]

            # ---- attention pools (coexist with phase 1: 2 p1 + 4 st +
            # 2 at = 8 psum banks; p3 opens after phase 1) ------------------
            p2sb = ctx.enter_context(tc.tile_pool(name="p2sb", bufs=4))
            p2est = ctx.enter_context(tc.tile_pool(name="p2est", bufs=12))
            p2cs = ctx.enter_context(tc.tile_pool(name="p2cs", bufs=3))
            p2st = ctx.enter_context(
                tc.tile_pool(name="p2st", bufs=4, space="PSUM"))
            p2at = ctx.enter_context(
                tc.tile_pool(name="p2at", bufs=2, space="PSUM"))
            p3ps = None    # assigned after phase 1 (bank budget)
            p3sb = None

            pending_norm = []
            pending_oproj = []
            slot = [0]
            in_p1 = [True]

            def drain_slot():
                slot[0] += 1
                while pending_norm:
                    pending_norm.pop(0)()     # norms gate psum reuse: ASAP
                if in_p1[0]:
                    return
                if pending_oproj and (len(pending_oproj) > 16
                                      or slot[0] % 2 == 0):
                    pending_oproj.pop(0)()

            def make_norm(at_ps, colsum, h, qc):
                def norm():
                    rep_ps = p2st.tile([P, SQ], F32, tag="stps",
                                       name="rep_ps")
                    nc.tensor.matmul(rep_ps[:], ones128_b[:], colsum[:],
                                     start=True, stop=True)
                    rep_sb = p2sb.tile([P, SQ], F32, tag="repsb")
                    nc.vector.reciprocal_approx_fast(rep_sb[:], rep_ps[:])
                    nc.vector.tensor_mul(attn_c[qc][:, h, :],
                                         at_ps[:], rep_sb[:])
                return norm

            def make_oproj(sc, ot):
                # two half-groups drained on consecutive slots: 432ns
                # tensor bursts interleave with attention blocks more
                # smoothly than a monolithic 864ns group
                state = {}

                def half_a():
                    ps = p3ps.tile([P, SQ], F32, tag="p3ps")
                    state["ps"] = ps
                    for kb in (0, 1):
                        nc.tensor.matmul(
                            ps[:], wo_r[:, kb, ot * P:(ot + 1) * P],
                            attn_c[sc][:, kb, :],
                            start=(kb == 0), stop=False)

                def half_b():
                    ps = state["ps"]
                    for kb in (2, 3):
                        nc.tensor.matmul(
                            ps[:], wo_r[:, kb, ot * P:(ot + 1) * P],
                            attn_c[sc][:, kb, :],
                            start=False, stop=(kb == 3))
                    # psum->sbuf f16 stage copies: GpSimd can't read PSUM,
                    # so split 3:1 DVE:ACT (ACT is the busier engine here —
                    # it owns all the exps)
                    stage = p3sb.tile([P, SQ], F16, tag="p3stage")
                    if ot % 4 == 0:
                        nc.scalar.copy(_one(stage[:]), obs_act[:])
                        nc.scalar.copy(stage[:], ps[:])
                    else:
                        dve_war_touch(stage)
                        nc.vector.tensor_copy(stage[:], ps[:])
                    nc.sync.dma_start(
                        outt.ap()[:, ot, sc * SQ:(sc + 1) * SQ],
                        stage[:])
                return half_a, half_b

            def att_unit(h, qc):
                """Generator: one causal-attention unit, yielding after
                each key-block so it can be pumped between phase-1
                projection groups (whose matmuls hide the exp/mask
                latency)."""
                nkb = 4 * (qc + 1)
                qs = qc * SQ
                at_ps = p2at.tile([P, SQ], F32, tag="atps")
                colsum = p2cs.tile([P, SQ], BF, tag="colsum")
                st_tiles = {}

                def off_of(kb):
                    return max(0, kb * P - qs)

                def emit_st(kb):
                    st_ps = p2st.tile([P, SQ], F32, tag="stps")
                    off = off_of(kb)
                    nc.tensor.matmul(
                        st_ps[:, off:SQ],
                        qk_c[kb // 4][:, NH_LOCAL + h,
                                      (kb % 4) * P:(kb % 4 + 1) * P],
                        qk_c[qc][:, h, off:SQ],
                        start=True, stop=True)
                    st_tiles[kb] = st_ps

                emit_st(0)
                emit_st(1)
                if nkb > 2:
                    emit_st(2)
                for kb in range(nkb):
                    drain_slot()
                    if kb + 3 < nkb:
                        emit_st(kb + 3)
                    st_ps = st_tiles.pop(kb)
                    off = off_of(kb)
                    est = p2est.tile([P, SQ], BF, tag="est")
                    nc.scalar.activation(est[:, off:SQ], st_ps[:, off:SQ],
                                         AF.Exp, scale=SCALE)
                    diag = kb * P + P - 1 > qs
                    if diag:  # crosses the causal diagonal: mask on DVE
                        nc.vector.tensor_mul(est[:, off:SQ], est[:, off:SQ],
                                             mask_t[:, 0:SQ - off])
                    # colsum accumulation: split across GpSimd (slow but
                    # otherwise idle, ~1150ns/op) and DVE so neither queue
                    # backs up; diagonal blocks stay on DVE behind their
                    # own mask multiply
                    if kb == 0:
                        eng = nc.gpsimd if not diag else nc.vector
                        eng.tensor_copy(colsum[:], est[:])
                    elif diag or kb % 2 == 0:
                        nc.vector.tensor_add(colsum[:, off:SQ],
                                             colsum[:, off:SQ],
                                             est[:, off:SQ])
                    else:
                        nc.gpsimd.tensor_add(colsum[:, off:SQ],
                                             colsum[:, off:SQ],
                                             est[:, off:SQ])
                    nc.tensor.matmul(
                        at_ps[:, off:SQ],
                        v_c[kb // 4][:, kb % 4, h * P:(h + 1) * P],
                        est[:, off:SQ],
                        start=(kb == 0), stop=(kb == nkb - 1))
                    yield
                pending_norm.append(make_norm(at_ps, colsum, h, qc))

            gens = [(h, qc, att_unit(h, qc))
                    for qc in range(NQC) for h in range(NH_LOCAL)]
            gen_idx = [0]

            def pump(max_qc):
                """Advance the attention emission by one key-block."""
                while gen_idx[0] < len(gens):
                    h, qc, g = gens[gen_idx[0]]
                    if qc >= max_qc:
                        return False
                    try:
                        next(g)
                        return True
                    except StopIteration:
                        if h == NH_LOCAL - 1:
                            for ot in range(KO):
                                pending_oproj.extend(make_oproj(qc, ot))
                        gen_idx[0] += 1
                return False

            # ================= phase 1: QKV projection =====================
            # fp16 matmuls, one pass over x in 512-token chunks.  After
            # each projection group, attention units whose inputs are
            # ready are pumped in (their exp/mask latency hides under the
            # next group's matmuls).  Chunk 0 is processed in two
            # 256-token halves so the first accumulation group only waits
            # for ~1MB of DMA, not 4MB.
            # w free layout: [0:512]=Q, [512:1024]=K, [1024:1536]=V
            p3w = ctx.enter_context(tc.tile_pool(name="p3w", bufs=1))
            wo_r = p3w.tile([P, NH_LOCAL, S], F16)
            with tc.tile_pool(name="p1w", bufs=1) as p1w, \
                 tc.tile_pool(name="p1x", bufs=2) as p1x, \
                 tc.tile_pool(name="p1ps", bufs=2, space="PSUM") as p1ps:

                w_r = p1w.tile([P, KO, 3 * NH_LOCAL * P], F16, tag="wr")
                x_tiles = []
                x_r0 = p1x.tile([P, KO, XCH], F16, tag="xr", name="xr0")
                # per-ko staging so the first accumulation group can
                # start as soon as ko-chunk 0 has landed (first half of
                # chunk 0 only; second half follows as one DMA)
                # initial loads fan out across engine queues (each
                # DMA_DIRECT2D dispatch costs ~600-850ns of queue time) and
                # only the lead-in-critical tiles go now: the first-half x
                # (contiguous xh0 copy) + wq.  wk/wv/x-2nd-half/wo are
                # dispatched behind the first projection groups below so
                # they don't steal DMA bandwidth from the critical path.
                HX = XCH // 2
                for kq in range(8):
                    ks = slice(2 * kq, 2 * (kq + 1))
                    nc.sync.dma_start(x_r0[:, ks, 0:HX], xh0.ap()[:, ks])
                    nc.gpsimd.dma_start(w_r[:, ks, 0:4 * P], wq.ap()[:, ks])
                nc.scalar.dma_start(w_r[:, :, 4 * P:8 * P], wk.ap())
                nc.scalar.dma_start(w_r[:, :, 8 * P:12 * P], wv.ap())
                nc.scalar.dma_start(x_r0[:, :, HX:XCH],
                                    xt.ap()[:, 0, :, HX:XCH])
                nc.gpsimd.dma_start(wo_r[:], wo.ap())
                x_tiles.append(x_r0)

                def qk_group(xc, x_r, ot, lo, hi):
                    ps = p1ps.tile([P, XCH], F32, tag="p1", name="ps")
                    for k in range(KO):
                        nc.tensor.matmul(
                            ps[:, 0:hi - lo], w_r[:, k, ot * P:(ot + 1) * P],
                            x_r[:, k, lo:hi], start=(k == 0),
                            stop=(k == KO - 1))
                    if ot % 2 == 0:
                        nc.scalar.copy(qk_c[xc][:, ot, lo:hi],
                                       ps[:, 0:hi - lo])
                    else:
                        nc.vector.tensor_copy(qk_c[xc][:, ot, lo:hi],
                                              ps[:, 0:hi - lo])
                    pump(xc)
                    pump(xc)

                def v_group(xc, x_r, st):
                    # V: out [s_tile(128), d(512)] — copies on ACT
                    ps = p1ps.tile([P, NH_LOCAL * P], F32, tag="p1",
                                   name="ps")
                    for k in range(KO):
                        nc.tensor.matmul(
                            ps[:], x_r[:, k, st * P:(st + 1) * P],
                            w_r[:, k, 2 * NH_LOCAL * P:3 * NH_LOCAL * P],
                            start=(k == 0), stop=(k == KO - 1))
                    nc.scalar.copy(v_c[xc][:, st, :], ps[:])
                    pump(xc)
                    pump(xc)

                for xc in range(NXCH):
                    if xc > 0:
                        x_r = p1x.tile([P, KO, XCH], F16, tag="xr")
                        nc.sync.dma_start(x_r[:], xt.ap()[:, xc])
                    else:
                        x_r = x_tiles[0]

                    if xc == 0:
                        for half in range(2):
                            lo, hi = half * HX, (half + 1) * HX
                            for ot in range(2 * NH_LOCAL):
                                qk_group(xc, x_r, ot, lo, hi)
                            for st in (2 * half, 2 * half + 1):
                                v_group(xc, x_r, st)
                    else:
                        for ot in range(2 * NH_LOCAL):
                            qk_group(xc, x_r, ot, 0, XCH)
                        for st in range(XCH // P):
                            v_group(xc, x_r, st)

            # ============ phase 2+3: remaining attention + o_proj ==========
            in_p1[0] = False
            p3ps = ctx.enter_context(
                tc.tile_pool(name="p3ps", bufs=2, space="PSUM"))
            p3sb = ctx.enter_context(tc.tile_pool(name="p3sb", bufs=4))

            while pump(NQC):
                pass
            while pending_norm:
                pending_norm.pop(0)()
            while pending_oproj:
                pending_oproj.pop(0)()

    from concourse.library_overlay import lower_extended_insts
    lower_extended_insts(nc)   # populate .instr bytes for custom ISA ops
    _split_multi_waits(nc)
    return nc


_NC_CACHE = None


def _get_nc():
    global _NC_CACHE
    if _NC_CACHE is None:
        _NC_CACHE = build()
    return _NC_CACHE


def _prep_inputs(hidden_states, w_qkv, w_o):
    """Host-side shard + pre-tile + fp16-cast for the 8 cores."""
    F16_NP = np.float16
    hidden_states = np.asarray(hidden_states, dtype=np.float32)
    w_qkv = np.asarray(w_qkv, dtype=np.float32)
    w_o = np.asarray(w_o, dtype=np.float32)
    B = hidden_states.shape[0]

    in_maps = []
    xt_by_b = {}
    xh0_by_b = {}
    for b in range(B):
        # xt[p, xc, ko, s'] = hidden[b, xc*512+s', ko*128+p]
        xt_by_b[b] = np.ascontiguousarray(
            hidden_states[b].T.reshape(KO, P, NXCH, XCH)
            .transpose(1, 2, 0, 3)
        ).astype(F16_NP)
        # first 256 tokens again, ko-contiguous (lead-in DMA)
        xh0_by_b[b] = np.ascontiguousarray(
            xt_by_b[b][:, 0, :, 0:XCH // 2])
    for c in range(8):
        b = c // 4
        hs = [4 * (c % 4) + j for j in range(NH_LOCAL)]
        q_rows = np.concatenate([np.arange(h * P, (h + 1) * P) for h in hs])
        k_rows = q_rows + H
        v_rows = q_rows + 2 * H

        def wtile(rows):
            # [p, ko, o] = w_qkv[rows[o], ko*128+p]
            w = w_qkv[rows, :]                      # [512, 2048]
            return np.ascontiguousarray(
                w.T.reshape(KO, P, len(rows)).transpose(1, 0, 2)
            ).astype(F16_NP)

        # wo[p, kb, o] = w_o[o, cols[kb*128+p]]
        wo_c = np.ascontiguousarray(
            w_o[:, q_rows].T.reshape(NH_LOCAL, P, S).transpose(1, 0, 2)
        ).astype(F16_NP)
        in_maps.append({
            "xt": xt_by_b[b],
            "xh0": xh0_by_b[b],
            "wq": wtile(q_rows),
            "wk": wtile(k_rows),
            "wv": wtile(v_rows),
            "wo": wo_c,
        })
    return in_maps


def run(hidden_states, w_qkv, w_o, trace=False, trace_cores=None):
    in_maps = _prep_inputs(hidden_states, w_qkv, w_o)
    nc = _get_nc()
    kwargs = {}
    if trace:
        kwargs["trace_cores"] = (trace_cores if trace_cores is not None
                                 else list(range(8)))
    res = run_bass_kernel_spmd(nc, in_maps, core_ids=list(range(8)),
                               trace=trace, **kwargs)
    B, S_, H_ = np.asarray(hidden_states).shape
    out = np.zeros((B, S_, H_), dtype=np.float32)
    for c in range(8):
        b = c // 4
        outt = res.results[c]["outt"]               # [128, 16, 2048] fp16
        outT = outt.astype(np.float32).transpose(1, 0, 2).reshape(H_, S_)
        out[b] += outT.T
    return out, res


def kernel(hidden_states, w_qkv, w_o):
    out, _ = run(hidden_states, w_qkv, w_o, trace=False)
    return out

